# revision 16
# baseline (speedup 1.0000x reference)
"""HaarWavelet2D (level=2) Trainium2 kernel, v2.

Contract: kernel(x, level) with x [8, 64, 256, 256] fp32, level=2.
Returns (low_freq, high_freq), each [8, 64, 256, 256] fp32 — matching the
jax reference (2-level Haar decomposition with bilinear resizes).

Sharding: data-parallel over batch — core b processes x[b] (64 channels).

v2 layout (validated in model2.py): rows-in-partitions, row-PARITY tiles
everywhere (row = 2*partition + p).  DRAM tensors are declared
[C, 128, 2, 256] so each G-channel group moves with ONE large DMA
(2 KiB descriptors).  Per group:
  s/d   = column-pair sum/diff, both parities in one TT (f32 in, bf16 out)
  ad    = |d| on the scalar engine; row-shifted (sE, adE) via one
          SBUF->SBUF DMA of an interleaved (s,ad) tile
  t1/m/ch0 = row-pair ops, bf16 TT
  vertical resizes = TensorE matmuls (parity-split banded matrices) in
          Gp=2 chunks, PSUM drained to bf16 SBUF by the scalar engine
  horizontal 255->256 = direct misaligned-bf16 TT diff + W0 mult + add
  level-1 repeats at half size; 128->256 horizontal via diffL tile and
  scalar_tensor_tensor writes straight into the f32 output tiles
Engine assignment (vector/scalar/gpsimd) per op class is parameterized
for rebalancing.
"""

import sys

if "/opt/trn_rl_repo" not in sys.path:
    sys.path.insert(0, "/opt/trn_rl_repo")

import numpy as np
import ml_dtypes

BF = ml_dtypes.bfloat16

B_, C_, H_, W_ = 8, 64, 256, 256
NCORES = 8
G = 4            # channels per group
GP0 = 2          # channels per level-0 matmul chunk
GP1 = 4          # channels per level-1 matmul chunk


# ----------------------------------------------------------------------------
# host-side weight construction
# ----------------------------------------------------------------------------

def _resize_matrix(n, N):
    M = np.zeros((N, n), dtype=np.float64)
    for i in range(N):
        c = (i + 0.5) * n / N - 0.5
        j0 = int(np.floor(c))
        f = c - j0
        M[i, min(max(j0, 0), n - 1)] += 1.0 - f
        M[i, min(max(j0 + 1, 0), n - 1)] += f
    return M


def _build_weights():
    V255 = _resize_matrix(255, 256)
    V128 = _resize_matrix(128, 256)
    Sv1 = np.zeros((255, 256))
    for r in range(255):
        Sv1[r, r] = 1.0
        Sv1[r, r + 1] = 1.0
    Va = 0.25 * (V255 @ Sv1)      # [256, 256]
    Vh = 0.25 * V255              # [256, 255]
    Vq = 0.25 * V128              # [256, 128]
    W0 = np.array([V255[i, i - 1] if i >= 1 else 0.0 for i in range(256)])

    w = {}
    for p in range(2):
        w[f"w_va_{p}e"] = Va[p::2, 0::2].T        # [128,128]
        w[f"w_va_{p}o"] = Va[p::2, 1::2].T        # [128,128]
        w[f"w_vh_{p}e"] = Vh[p::2, 0::2].T        # [128,128]
        w[f"w_vh_{p}o"] = Vh[p::2, 1::2].T        # [127,128]
        w[f"w_vq_{p}"] = Vq[p::2, :].T            # [128,128]
    w["w0t"] = np.tile(W0[None, :], (128, 1))     # [128,256]
    w["w0full"] = np.tile(W0[None, None, None, :], (128, 2, G, 1)).reshape(
        128, 2 * G * 256)                          # [128, 2*G*256]
    return {k: v.astype(BF) for k, v in w.items()}


_WEIGHTS = None


def _weights():
    global _WEIGHTS
    if _WEIGHTS is None:
        _WEIGHTS = _build_weights()
    return _WEIGHTS


# ----------------------------------------------------------------------------
# bass program
# ----------------------------------------------------------------------------

_NC_CACHE = {}


def build_nc(C=C_):
    key = C
    if key in _NC_CACHE:
        return _NC_CACHE[key]

    import concourse.bass as bass
    import concourse.bacc as bacc
    import concourse.tile as tile
    import concourse.mybir as mybir

    F32 = mybir.dt.float32
    BF16 = mybir.dt.bfloat16
    Alu = mybir.AluOpType
    Act = mybir.ActivationFunctionType
    P = 128

    nc = bacc.Bacc("TRN2", target_bir_lowering=False)
    # [C, 256, 256] viewed as [C, 128, 2, 256]: row = 2r + p
    x_d = nc.dram_tensor("x", [C, P, 2, W_], F32, kind="ExternalInput")
    wt = _weights()
    w_d = {
        name: nc.dram_tensor(name, list(arr.shape), BF16, kind="ExternalInput")
        for name, arr in wt.items()
    }
    low_d = nc.dram_tensor("low", [C, P, 2, W_], BF16, kind="ExternalOutput")
    high_d = nc.dram_tensor("high", [C, P, 2, W_], BF16, kind="ExternalOutput")

    with tile.TileContext(nc) as tc:
        with (
            tc.tile_pool(name="consts", bufs=1) as consts,
            tc.tile_pool(name="xin", bufs=3) as xin,
            tc.tile_pool(name="front", bufs=2) as front,
            tc.tile_pool(name="qp", bufs=1) as qp,
            tc.tile_pool(name="hor", bufs=2) as hor,
            tc.tile_pool(name="lv1", bufs=2) as lv1,
            tc.tile_pool(name="outp", bufs=2) as outp,
            tc.tile_pool(name="ps0", bufs=1, space="PSUM") as ps0,
        ):
            wtile = {}
            for name, arr in wt.items():
                t = consts.tile(list(arr.shape), BF16, tag=name)
                nc.sync.dma_start(out=t, in_=w_d[name][:, :])
                wtile[name] = t

            # persistent tiles (bufs=1 qp pool -> stable addresses for the
            # one-time edge memsets; values cancel algebraically, the memset
            # only guards against NaN garbage)
            qL2 = qp.tile([P, 2, 2, G, 256], BF16, tag="qL")
            qh2 = qp.tile([P, 2, 2, G, 256], BF16, tag="qh")
            diffT_L2 = qp.tile([P, 2, 2, G, 256], BF16, tag="diffT_L")
            diffT_h2 = qp.tile([P, 2, 2, G, 256], BF16, tag="diffT_h")
            qlo2 = qp.tile([P, 2, 2, G, 128], BF16, tag="qlo")
            qh12 = qp.tile([P, 2, 2, G, 128], BF16, tag="qh1")
            diffL_lo2 = qp.tile([P, 2, 2, G, 129], BF16, tag="diffL_lo")
            diffL_h12 = qp.tile([P, 2, 2, G, 129], BF16, tag="diffL_h1")

            nc.vector.memset(qL2[:, :, :, :, 255:256], 0.0)
            nc.vector.memset(qh2[:, :, :, :, 255:256], 0.0)
            nc.vector.memset(diffT_L2[:, :, :, :, 0:1], 0.0)
            nc.vector.memset(diffT_h2[:, :, :, :, 0:1], 0.0)
            for dl in (diffL_lo2, diffL_h12):
                nc.vector.memset(dl[:, :, :, :, 0:1], 0.0)
                nc.vector.memset(dl[:, :, :, :, 128:129], 0.0)

            w0full = wtile["w0full"][:, :].rearrange(
                "r (p g w) -> r p g w", p=2, g=G)

            n_iter = C // G

            def stage_load(it):
                c0 = it * G
                X = xin.tile([P, G, 2, 256], BF16, tag="X")
                nc.gpsimd.dma_start(
                    out=X, in_=x_d[c0:c0 + G].rearrange("c r p w -> r c p w"))
                return X

            def stage_a(it, X):
                """level-0 elementwise, level-0 matmuls + drains."""
                t = {}
                c0 = it * G
                sl = it % 2
                qL, qh = qL2[:, sl], qh2[:, sl]
                t["qL"], t["qh"] = qL, qh

                SDA = front.tile([P, 4, G, 255], BF16, tag="SDA")
                d0 = front.tile([P, 2, G, 255], BF16, tag="d0")
                Xp = X[:, :, :, 0:255].rearrange("r g p w -> r p g w")
                Xp1 = X[:, :, :, 1:256].rearrange("r g p w -> r p g w")
                nc.vector.tensor_tensor(
                    out=SDA[:, 0:2], in0=Xp, in1=Xp1, op=Alu.add)
                nc.vector.tensor_tensor(
                    out=d0, in0=Xp, in1=Xp1, op=Alu.subtract)
                nc.scalar.activation(out=SDA[:, 2:4], in_=d0, func=Act.Abs)

                SDA2 = front.tile([127, 2, G, 255], BF16, tag="SDA2")
                nc.sync.dma_start(out=SDA2, in_=SDA[1:128, 0:4:2])

                t1 = front.tile([P, 2, G, 255], BF16, tag="t1")
                m0 = front.tile([P, 2, G, 255], BF16, tag="m0")
                a1 = front.tile([P, 2, G, 255], BF16, tag="a1")
                ch0 = front.tile([P, 2, G, 255], BF16, tag="ch0")
                nc.vector.tensor_tensor(
                    out=t1[:, 0], in0=SDA[:, 0], in1=SDA[:, 1], op=Alu.subtract)
                nc.vector.tensor_tensor(
                    out=t1[0:127, 1], in0=SDA[0:127, 1], in1=SDA2[:, 0],
                    op=Alu.subtract)
                nc.vector.tensor_tensor(
                    out=m0[:, 0], in0=SDA[:, 2], in1=SDA[:, 3], op=Alu.max)
                nc.vector.tensor_tensor(
                    out=m0[0:127, 1], in0=SDA[0:127, 3], in1=SDA2[:, 1],
                    op=Alu.max)
                nc.scalar.activation(
                    out=a1[:, 0], in_=t1[:, 0], func=Act.Abs, scale=0.5)
                nc.scalar.activation(
                    out=a1[0:127, 1], in_=t1[0:127, 1], func=Act.Abs, scale=0.5)
                nc.vector.tensor_tensor(
                    out=ch0[:, 0], in0=a1[:, 0], in1=m0[:, 0], op=Alu.add)
                nc.vector.tensor_tensor(
                    out=ch0[0:127, 1], in0=a1[0:127, 1], in1=m0[0:127, 1],
                    op=Alu.add)

                NF0 = GP0 * 255
                for ci in range(G // GP0):
                    cc = ci * GP0
                    Y_L = ps0.tile([P, 2, 512], F32, tag=f"Y_A{ci}")
                    Y_h = ps0.tile([P, 2, 512], F32, tag=f"Y_B{ci}")
                    for p in range(2):
                        nc.tensor.matmul(
                            out=Y_L[:, p, 0:NF0], lhsT=wtile[f"w_va_{p}e"][:, :],
                            rhs=SDA[:, 0, cc:cc + GP0, :], start=True, stop=False)
                        nc.tensor.matmul(
                            out=Y_L[:, p, 0:NF0], lhsT=wtile[f"w_va_{p}o"][:, :],
                            rhs=SDA[:, 1, cc:cc + GP0, :], start=False, stop=True)
                        nc.tensor.matmul(
                            out=Y_h[:, p, 0:NF0], lhsT=wtile[f"w_vh_{p}e"][:, :],
                            rhs=ch0[:, 0, cc:cc + GP0, :], start=True, stop=False)
                        nc.tensor.matmul(
                            out=Y_h[:, p, 0:NF0], lhsT=wtile[f"w_vh_{p}o"][:, :],
                            rhs=ch0[0:127, 1, cc:cc + GP0, :], start=False,
                            stop=True)
                    nc.scalar.copy(
                        out=qL[:, :, cc:cc + GP0, 0:255],
                        in_=Y_L[:, :, 0:NF0].rearrange(
                            "r p (g w) -> r p g w", w=255))
                    nc.scalar.copy(
                        out=qh[:, :, cc:cc + GP0, 0:255],
                        in_=Y_h[:, :, 0:NF0].rearrange(
                            "r p (g w) -> r p g w", w=255))
                return t

            def stage_b(it, t):
                """level-0 horizontal resize, level-1 front + matmuls + drains."""
                sl = it % 2
                qL, qh = t["qL"], t["qh"]
                diffT_L, diffT_h = diffT_L2[:, sl], diffT_h2[:, sl]
                qlo, qh1 = qlo2[:, sl], qh12[:, sl]

                L0x = hor.tile([P, 2, G, 256], BF16, tag="L0x")
                h0x = hor.tile([P, 2, G, 256], BF16, tag="h0x")
                tmpT = hor.tile([P, 2, G, 256], BF16, tag="tmpT")
                t["h0x"] = h0x
                for q, diffT, out in (
                    (qL, diffT_L, L0x),
                    (qh, diffT_h, h0x),
                ):
                    nc.vector.tensor_tensor(
                        out=diffT[:, :, :, 1:256], in0=q[:, :, :, 0:255],
                        in1=q[:, :, :, 1:256], op=Alu.subtract)
                    nc.vector.tensor_tensor(
                        out=tmpT, in0=diffT, in1=w0full, op=Alu.mult)
                    nc.vector.tensor_tensor(out=out, in0=q, in1=tmpT, op=Alu.add)

                s2 = lv1.tile([P, 2, G, 128], BF16, tag="s2")
                d2 = lv1.tile([P, 2, G, 128], BF16, tag="d2")
                ad2 = lv1.tile([P, 2, G, 128], BF16, tag="ad2")
                nc.gpsimd.tensor_tensor(
                    out=s2, in0=L0x[:, :, :, 0:256:2], in1=L0x[:, :, :, 1:256:2],
                    op=Alu.add)
                nc.gpsimd.tensor_tensor(
                    out=d2, in0=L0x[:, :, :, 0:256:2], in1=L0x[:, :, :, 1:256:2],
                    op=Alu.subtract)
                nc.scalar.activation(out=ad2, in_=d2, func=Act.Abs)
                lsum1 = lv1.tile([P, G, 128], BF16, tag="lsum1")
                t1b = lv1.tile([P, G, 128], BF16, tag="t1b")
                m1 = lv1.tile([P, G, 128], BF16, tag="m1")
                a1b = lv1.tile([P, G, 128], BF16, tag="a1b")
                ch1 = lv1.tile([P, G, 128], BF16, tag="ch1")
                nc.vector.tensor_tensor(
                    out=lsum1, in0=s2[:, 0], in1=s2[:, 1], op=Alu.add)
                nc.vector.tensor_tensor(
                    out=t1b, in0=s2[:, 0], in1=s2[:, 1], op=Alu.subtract)
                nc.vector.tensor_tensor(
                    out=m1, in0=ad2[:, 0], in1=ad2[:, 1], op=Alu.max)
                nc.scalar.activation(out=a1b, in_=t1b, func=Act.Abs, scale=0.5)
                nc.vector.tensor_tensor(
                    out=ch1, in0=a1b, in1=m1, op=Alu.add)

                NF1 = G * 128
                Y_lo = ps0.tile([P, 2, 512], F32, tag="Y_A0")
                Y_h1 = ps0.tile([P, 2, 512], F32, tag="Y_B0")
                for p in range(2):
                    nc.tensor.matmul(
                        out=Y_lo[:, p, 0:NF1], lhsT=wtile[f"w_vq_{p}"][:, :],
                        rhs=lsum1, start=True, stop=True)
                    nc.tensor.matmul(
                        out=Y_h1[:, p, 0:NF1], lhsT=wtile[f"w_vq_{p}"][:, :],
                        rhs=ch1, start=True, stop=True)
                nc.scalar.copy(
                    out=qlo,
                    in_=Y_lo[:, :, 0:NF1].rearrange("r p (g w) -> r p g w", w=128))
                nc.scalar.copy(
                    out=qh1,
                    in_=Y_h1[:, :, 0:NF1].rearrange("r p (g w) -> r p g w", w=128))
                return t

            def stage_c(it, t):
                """level-1 horizontal + finalize + stores."""
                c0 = it * G
                sl = it % 2
                qlo, qh1 = qlo2[:, sl], qh12[:, sl]
                diffL_lo, diffL_h1 = diffL_lo2[:, sl], diffL_h12[:, sl]
                h0x = t["h0x"]

                for q, dl in ((qlo, diffL_lo), (qh1, diffL_h1)):
                    nc.vector.tensor_tensor(
                        out=dl[:, :, :, 1:128], in0=q[:, :, :, 0:127],
                        in1=q[:, :, :, 1:128], op=Alu.subtract)

                lowT = outp.tile([P, G, 2, 256], BF16, tag="lowT")
                highT = outp.tile([P, G, 2, 256], BF16, tag="highT")
                hh = lv1.tile([P, 2, G, 256], BF16, tag="hh")
                for p in range(2):
                    nc.vector.scalar_tensor_tensor(
                        out=lowT[:, :, p, 0:256:2], in0=diffL_lo[:, p, :, 0:128],
                        scalar=0.25, in1=qlo[:, p], op0=Alu.mult, op1=Alu.add)
                    nc.vector.scalar_tensor_tensor(
                        out=lowT[:, :, p, 1:256:2], in0=diffL_lo[:, p, :, 1:129],
                        scalar=-0.25, in1=qlo[:, p], op0=Alu.mult, op1=Alu.add)
                    nc.vector.scalar_tensor_tensor(
                        out=hh[:, p, :, 0:256:2], in0=diffL_h1[:, p, :, 0:128],
                        scalar=0.25, in1=qh1[:, p], op0=Alu.mult, op1=Alu.add)
                    nc.vector.scalar_tensor_tensor(
                        out=hh[:, p, :, 1:256:2], in0=diffL_h1[:, p, :, 1:129],
                        scalar=-0.25, in1=qh1[:, p], op0=Alu.mult, op1=Alu.add)
                nc.gpsimd.tensor_tensor(
                    out=highT.rearrange("r c p w -> r p c w"), in0=hh, in1=h0x,
                    op=Alu.add)

                nc.sync.dma_start(
                    out=low_d[c0:c0 + G].rearrange("c r p w -> r c p w"),
                    in_=lowT)
                nc.sync.dma_start(
                    out=high_d[c0:c0 + G].rearrange("c r p w -> r c p w"),
                    in_=highT)

            # plain order with loads prefetched 2 groups ahead
            xt = {0: stage_load(0), 1: stage_load(1)}
            for it in range(n_iter):
                if it + 2 < n_iter:
                    xt[it + 2] = stage_load(it + 2)
                t = stage_a(it, xt.pop(it))
                stage_b(it, t)
                stage_c(it, t)

    nc.compile()
    _NC_CACHE[key] = nc
    return nc


# ----------------------------------------------------------------------------
# host entry points
# ----------------------------------------------------------------------------

_RUNNER = None


def _get_runner():
    """Builds (once) a cached sharded jit executable over the 8 cores."""
    global _RUNNER
    if _RUNNER is not None:
        return _RUNNER

    import jax
    from jax.sharding import Mesh, PartitionSpec, NamedSharding
    from jax.experimental.shard_map import shard_map
    import concourse.mybir as mybir
    from concourse import bass2jax
    from concourse.bass2jax import _bass_exec_p, partition_id_tensor

    bass2jax.install_neuronx_cc_hook()
    nc = build_nc(C_)

    partition_name = nc.partition_id_tensor.name if nc.partition_id_tensor else None
    in_names, out_names, out_avals = [], [], []
    for alloc in nc.m.functions[0].allocations:
        if not isinstance(alloc, mybir.MemoryLocationSet):
            continue
        name = alloc.memorylocations[0].name
        if alloc.kind == "ExternalInput":
            if name != partition_name:
                in_names.append(name)
        elif alloc.kind == "ExternalOutput":
            out_names.append(name)
            out_avals.append(jax.core.ShapedArray(
                tuple(alloc.tensor_shape), mybir.dt.np(alloc.dtype)))
    n_params = len(in_names)
    all_in_names = list(in_names) + list(out_names)
    if partition_name is not None:
        all_in_names.append(partition_name)

    def _body(*args):
        operands = list(args)
        if partition_name is not None:
            operands.append(partition_id_tensor())
        return tuple(_bass_exec_p.bind(
            *operands,
            out_avals=tuple(out_avals),
            in_names=tuple(all_in_names),
            out_names=tuple(out_names),
            lowering_input_output_aliases=(),
            sim_require_finite=True,
            sim_require_nnan=True,
            nc=nc,
        ))

    devices = jax.devices()[:NCORES]
    mesh = Mesh(np.asarray(devices), ("core",))
    n_in = n_params + len(out_names)
    sharded = jax.jit(shard_map(
        _body, mesh=mesh,
        in_specs=(PartitionSpec("core"),) * n_in,
        out_specs=(PartitionSpec("core"),) * len(out_names),
        check_rep=False))

    shard0 = NamedSharding(mesh, PartitionSpec("core"))
    wt = _weights()
    static = {}
    for name in in_names:
        if name == "x":
            continue
        arr = np.concatenate([wt[name]] * NCORES, axis=0)
        static[name] = jax.device_put(arr, shard0)
    for name, aval in zip(out_names, out_avals):
        z = np.zeros((aval.shape[0] * NCORES,) + tuple(aval.shape[1:]),
                     dtype=aval.dtype)
        static[name] = jax.device_put(z, shard0)

    def run(x_global):
        ops = []
        for name in in_names:
            ops.append(x_global if name == "x" else static[name])
        for name in out_names:
            ops.append(static[name])
        outs = sharded(*ops)
        return dict(zip(out_names, outs))

    _RUNNER = (run, shard0)
    return _RUNNER


def _run_device(x, trace=False):
    """x: [8, 64, 256, 256] fp32. Returns (low, high, results_obj)."""
    if trace:
        from concourse import bass_utils
        nc = build_nc(C_)
        wt = _weights()
        in_maps = [
            dict(wt, x=np.ascontiguousarray(x[b]).reshape(C_, 128, 2, W_))
            for b in range(NCORES)
        ]
        res = bass_utils.run_bass_kernel_spmd(
            nc, in_maps, core_ids=list(range(NCORES)), trace=True)
        low = np.stack([
            res.results[b]["low"].reshape(C_, H_, W_) for b in range(NCORES)])
        high = np.stack([
            res.results[b]["high"].reshape(C_, H_, W_) for b in range(NCORES)])
        return low.astype(np.float32), high.astype(np.float32), res

    run, _ = _get_runner()
    outs = run(np.ascontiguousarray(x).reshape(B_ * C_, 128, 2, W_))
    low = np.asarray(outs["low"]).astype(np.float32).reshape(B_, C_, H_, W_)
    high = np.asarray(outs["high"]).astype(np.float32).reshape(B_, C_, H_, W_)
    return low, high, None


def _fallback(x, level):
    """Numpy port of the reference for unexpected shapes/levels."""
    xl = x.astype(np.float64)
    low = xl
    high = np.zeros_like(xl)
    Bb, Cc, H, W = xl.shape

    def up(a, n_r, n_c):
        Mr = _resize_matrix(a.shape[-2], n_r)
        Mc = _resize_matrix(a.shape[-1], n_c)
        return np.einsum("ij,...jk,lk->...il", Mr, a, Mc)

    for lv in range(level):
        stride = 2 ** lv
        if H // stride < 2 or W // stride < 2:
            break
        x00 = low[..., 0:H - 1:stride, 0:W - 1:stride]
        x01 = low[..., 0:H - 1:stride, 1:W:stride]
        x10 = low[..., 1:H:stride, 0:W - 1:stride]
        x11 = low[..., 1:H:stride, 1:W:stride]
        ll = (x00 + x01 + x10 + x11) * 0.25
        lh = (x00 + x01 - x10 - x11) * 0.25
        hl = (x00 - x01 + x10 - x11) * 0.25
        hh = (x00 - x01 - x10 + x11) * 0.25
        ch = np.abs(lh) + np.abs(hl) + np.abs(hh)
        high = high + up(ch, H, W)
        low = up(ll, H, W)
    if level > 0:
        high = high / level
    return low.astype(np.float32), high.astype(np.float32)


def kernel(x, level):
    x = np.asarray(x, dtype=np.float32)
    level = int(level)
    if level != 2 or x.shape != (B_, C_, H_, W_):
        return _fallback(x, level)
    low, high, _ = _run_device(x)
    return low, high


# revision 17
# speedup vs baseline: 1.0983x; 1.0983x over previous
"""HaarWavelet2D (level=2) Trainium2 kernel, v2.

Contract: kernel(x, level) with x [8, 64, 256, 256] fp32, level=2.
Returns (low_freq, high_freq), each [8, 64, 256, 256] fp32 — matching the
jax reference (2-level Haar decomposition with bilinear resizes).

Sharding: data-parallel over batch — core b processes x[b] (64 channels).

v2 layout (validated in model2.py): rows-in-partitions, row-PARITY tiles
everywhere (row = 2*partition + p).  DRAM tensors are declared
[C, 128, 2, 256] so each G-channel group moves with ONE large DMA
(2 KiB descriptors).  Per group:
  s/d   = column-pair sum/diff, both parities in one TT (f32 in, bf16 out)
  ad    = |d| on the scalar engine; row-shifted (sE, adE) via one
          SBUF->SBUF DMA of an interleaved (s,ad) tile
  t1/m/ch0 = row-pair ops, bf16 TT
  vertical resizes = TensorE matmuls (parity-split banded matrices) in
          Gp=2 chunks, PSUM drained to bf16 SBUF by the scalar engine
  horizontal 255->256 = direct misaligned-bf16 TT diff + W0 mult + add
  level-1 repeats at half size; 128->256 horizontal via diffL tile and
  scalar_tensor_tensor writes straight into the f32 output tiles
Engine assignment (vector/scalar/gpsimd) per op class is parameterized
for rebalancing.
"""

import sys

if "/opt/trn_rl_repo" not in sys.path:
    sys.path.insert(0, "/opt/trn_rl_repo")

import numpy as np
import ml_dtypes

BF = ml_dtypes.bfloat16

B_, C_, H_, W_ = 8, 64, 256, 256
NCORES = 8
G = 4            # channels per group
GP0 = 2          # channels per level-0 matmul chunk
GP1 = 4          # channels per level-1 matmul chunk


# ----------------------------------------------------------------------------
# host-side weight construction
# ----------------------------------------------------------------------------

def _resize_matrix(n, N):
    M = np.zeros((N, n), dtype=np.float64)
    for i in range(N):
        c = (i + 0.5) * n / N - 0.5
        j0 = int(np.floor(c))
        f = c - j0
        M[i, min(max(j0, 0), n - 1)] += 1.0 - f
        M[i, min(max(j0 + 1, 0), n - 1)] += f
    return M


def _build_weights():
    V255 = _resize_matrix(255, 256)
    V128 = _resize_matrix(128, 256)
    Sv1 = np.zeros((255, 256))
    for r in range(255):
        Sv1[r, r] = 1.0
        Sv1[r, r + 1] = 1.0
    Va = 0.25 * (V255 @ Sv1)      # [256, 256]
    Vh = 0.25 * V255              # [256, 255]
    Vq = 0.25 * V128              # [256, 128]
    W0 = np.array([V255[i, i - 1] if i >= 1 else 0.0 for i in range(256)])

    w = {}
    for p in range(2):
        w[f"w_va_{p}e"] = Va[p::2, 0::2].T        # [128,128]
        w[f"w_va_{p}o"] = Va[p::2, 1::2].T        # [128,128]
        w[f"w_vh_{p}e"] = Vh[p::2, 0::2].T        # [128,128]
        w[f"w_vh_{p}o"] = Vh[p::2, 1::2].T        # [127,128]
        w[f"w_vq_{p}"] = Vq[p::2, :].T            # [128,128]
    w["w0t"] = np.tile(W0[None, :], (128, 1))     # [128,256]
    w["w0full"] = np.tile(W0[None, None, None, :], (128, 2, G, 1)).reshape(
        128, 2 * G * 256)                          # [128, 2*G*256]
    return {k: v.astype(BF) for k, v in w.items()}


_WEIGHTS = None


def _weights():
    global _WEIGHTS
    if _WEIGHTS is None:
        _WEIGHTS = _build_weights()
    return _WEIGHTS


# ----------------------------------------------------------------------------
# bass program
# ----------------------------------------------------------------------------

_NC_CACHE = {}


def build_nc(C=C_):
    key = C
    if key in _NC_CACHE:
        return _NC_CACHE[key]

    import concourse.bass as bass
    import concourse.bacc as bacc
    import concourse.tile as tile
    import concourse.mybir as mybir

    F32 = mybir.dt.float32
    BF16 = mybir.dt.bfloat16
    Alu = mybir.AluOpType
    Act = mybir.ActivationFunctionType
    P = 128

    nc = bacc.Bacc("TRN2", target_bir_lowering=False)
    # [C, 256, 256] viewed as [C, 128, 2, 256]: row = 2r + p
    x_d = nc.dram_tensor("x", [C, P, 2, W_], F32, kind="ExternalInput")
    wt = _weights()
    w_d = {
        name: nc.dram_tensor(name, list(arr.shape), BF16, kind="ExternalInput")
        for name, arr in wt.items()
    }
    low_d = nc.dram_tensor("low", [C, P, 2, W_], BF16, kind="ExternalOutput")
    high_d = nc.dram_tensor("high", [C, P, 2, W_], BF16, kind="ExternalOutput")

    with tile.TileContext(nc) as tc:
        with (
            tc.tile_pool(name="consts", bufs=1) as consts,
            tc.tile_pool(name="xin", bufs=4) as xin,
            tc.tile_pool(name="front", bufs=2) as front,
            tc.tile_pool(name="qp", bufs=1) as qp,
            tc.tile_pool(name="hor", bufs=2) as hor,
            tc.tile_pool(name="lv1", bufs=2) as lv1,
            tc.tile_pool(name="outp", bufs=2) as outp,
            tc.tile_pool(name="ps0", bufs=1, space="PSUM") as ps0,
        ):
            wtile = {}
            for name, arr in wt.items():
                t = consts.tile(list(arr.shape), BF16, tag=name)
                nc.sync.dma_start(out=t, in_=w_d[name][:, :])
                wtile[name] = t

            # persistent tiles (bufs=1 qp pool -> stable addresses for the
            # one-time edge memsets; values cancel algebraically, the memset
            # only guards against NaN garbage)
            qL2 = qp.tile([P, 2, 2, G, 256], BF16, tag="qL")
            qh2 = qp.tile([P, 2, 2, G, 256], BF16, tag="qh")
            diffT_L2 = qp.tile([P, 2, 2, G, 256], BF16, tag="diffT_L")
            diffT_h2 = qp.tile([P, 2, 2, G, 256], BF16, tag="diffT_h")
            qlo2 = qp.tile([P, 2, 2, G, 128], BF16, tag="qlo")
            qh12 = qp.tile([P, 2, 2, G, 128], BF16, tag="qh1")
            diffL_lo2 = qp.tile([P, 2, 2, G, 129], BF16, tag="diffL_lo")
            diffL_h12 = qp.tile([P, 2, 2, G, 129], BF16, tag="diffL_h1")

            nc.vector.memset(qL2[:, :, :, :, 255:256], 0.0)
            nc.vector.memset(qh2[:, :, :, :, 255:256], 0.0)
            nc.vector.memset(diffT_L2[:, :, :, :, 0:1], 0.0)
            nc.vector.memset(diffT_h2[:, :, :, :, 0:1], 0.0)
            for dl in (diffL_lo2, diffL_h12):
                nc.vector.memset(dl[:, :, :, :, 0:1], 0.0)
                nc.vector.memset(dl[:, :, :, :, 128:129], 0.0)

            w0full = wtile["w0full"][:, :].rearrange(
                "r (p g w) -> r p g w", p=2, g=G)

            n_iter = C // G

            def stage_load(it):
                c0 = it * G
                X = xin.tile([P, G, 2, 256], BF16, tag="X")
                nc.gpsimd.dma_start(
                    out=X, in_=x_d[c0:c0 + G].rearrange("c r p w -> r c p w"))
                return X

            def stage_a(it, X):
                """level-0 elementwise, level-0 matmuls + drains."""
                t = {}
                c0 = it * G
                sl = it % 2
                qL, qh = qL2[:, sl], qh2[:, sl]
                t["qL"], t["qh"] = qL, qh

                SDA = front.tile([P, 4, G, 255], BF16, tag="SDA")
                d0 = front.tile([P, 2, G, 255], BF16, tag="d0")
                Xp = X[:, :, :, 0:255].rearrange("r g p w -> r p g w")
                Xp1 = X[:, :, :, 1:256].rearrange("r g p w -> r p g w")
                nc.vector.tensor_tensor(
                    out=SDA[:, 0:2], in0=Xp, in1=Xp1, op=Alu.add)
                nc.vector.tensor_tensor(
                    out=d0, in0=Xp, in1=Xp1, op=Alu.subtract)
                nc.scalar.activation(out=SDA[:, 2:4], in_=d0, func=Act.Abs)

                SDA2 = front.tile([127, 2, G, 255], BF16, tag="SDA2")
                nc.sync.dma_start(out=SDA2, in_=SDA[1:128, 0:4:2])

                t1 = front.tile([P, 2, G, 255], BF16, tag="t1")
                m0 = front.tile([P, 2, G, 255], BF16, tag="m0")
                a1 = front.tile([P, 2, G, 255], BF16, tag="a1")
                ch0 = front.tile([P, 2, G, 255], BF16, tag="ch0")
                nc.vector.tensor_tensor(
                    out=t1[:, 0], in0=SDA[:, 0], in1=SDA[:, 1], op=Alu.subtract)
                nc.vector.tensor_tensor(
                    out=t1[0:127, 1], in0=SDA[0:127, 1], in1=SDA2[:, 0],
                    op=Alu.subtract)
                nc.vector.tensor_tensor(
                    out=m0[:, 0], in0=SDA[:, 2], in1=SDA[:, 3], op=Alu.max)
                nc.vector.tensor_tensor(
                    out=m0[0:127, 1], in0=SDA[0:127, 3], in1=SDA2[:, 1],
                    op=Alu.max)
                nc.scalar.activation(
                    out=a1[:, 0], in_=t1[:, 0], func=Act.Abs, scale=0.5)
                nc.scalar.activation(
                    out=a1[0:127, 1], in_=t1[0:127, 1], func=Act.Abs, scale=0.5)
                nc.vector.tensor_tensor(
                    out=ch0[:, 0], in0=a1[:, 0], in1=m0[:, 0], op=Alu.add)
                nc.vector.tensor_tensor(
                    out=ch0[0:127, 1], in0=a1[0:127, 1], in1=m0[0:127, 1],
                    op=Alu.add)

                NF0 = GP0 * 255
                for ci in range(G // GP0):
                    cc = ci * GP0
                    Y_L = ps0.tile([P, 2, 512], F32, tag=f"Y_A{ci}")
                    Y_h = ps0.tile([P, 2, 512], F32, tag=f"Y_B{ci}")
                    for p in range(2):
                        nc.tensor.matmul(
                            out=Y_L[:, p, 0:NF0], lhsT=wtile[f"w_va_{p}e"][:, :],
                            rhs=SDA[:, 0, cc:cc + GP0, :], start=True, stop=False)
                        nc.tensor.matmul(
                            out=Y_L[:, p, 0:NF0], lhsT=wtile[f"w_va_{p}o"][:, :],
                            rhs=SDA[:, 1, cc:cc + GP0, :], start=False, stop=True)
                        nc.tensor.matmul(
                            out=Y_h[:, p, 0:NF0], lhsT=wtile[f"w_vh_{p}e"][:, :],
                            rhs=ch0[:, 0, cc:cc + GP0, :], start=True, stop=False)
                        nc.tensor.matmul(
                            out=Y_h[:, p, 0:NF0], lhsT=wtile[f"w_vh_{p}o"][:, :],
                            rhs=ch0[0:127, 1, cc:cc + GP0, :], start=False,
                            stop=True)
                    nc.scalar.copy(
                        out=qL[:, :, cc:cc + GP0, 0:255],
                        in_=Y_L[:, :, 0:NF0].rearrange(
                            "r p (g w) -> r p g w", w=255))
                    nc.scalar.copy(
                        out=qh[:, :, cc:cc + GP0, 0:255],
                        in_=Y_h[:, :, 0:NF0].rearrange(
                            "r p (g w) -> r p g w", w=255))
                return t

            def stage_b(it, t):
                """level-0 horizontal resize, level-1 front + matmuls + drains."""
                sl = it % 2
                qL, qh = t["qL"], t["qh"]
                diffT_L, diffT_h = diffT_L2[:, sl], diffT_h2[:, sl]
                qlo, qh1 = qlo2[:, sl], qh12[:, sl]

                L0x = hor.tile([P, 2, G, 256], BF16, tag="L0x")
                h0x = hor.tile([P, 2, G, 256], BF16, tag="h0x")
                tmpT = hor.tile([P, 2, G, 256], BF16, tag="tmpT")
                t["h0x"] = h0x
                for q, diffT, out in (
                    (qL, diffT_L, L0x),
                    (qh, diffT_h, h0x),
                ):
                    nc.vector.tensor_tensor(
                        out=diffT[:, :, :, 1:256], in0=q[:, :, :, 0:255],
                        in1=q[:, :, :, 1:256], op=Alu.subtract)
                    nc.vector.tensor_tensor(
                        out=tmpT, in0=diffT, in1=w0full, op=Alu.mult)
                    nc.vector.tensor_tensor(out=out, in0=q, in1=tmpT, op=Alu.add)

                s2 = lv1.tile([P, 2, G, 128], BF16, tag="s2")
                d2 = lv1.tile([P, 2, G, 128], BF16, tag="d2")
                ad2 = lv1.tile([P, 2, G, 128], BF16, tag="ad2")
                nc.gpsimd.tensor_tensor(
                    out=s2, in0=L0x[:, :, :, 0:256:2], in1=L0x[:, :, :, 1:256:2],
                    op=Alu.add)
                nc.gpsimd.tensor_tensor(
                    out=d2, in0=L0x[:, :, :, 0:256:2], in1=L0x[:, :, :, 1:256:2],
                    op=Alu.subtract)
                nc.scalar.activation(out=ad2, in_=d2, func=Act.Abs)
                lsum1 = lv1.tile([P, G, 128], BF16, tag="lsum1")
                t1b = lv1.tile([P, G, 128], BF16, tag="t1b")
                m1 = lv1.tile([P, G, 128], BF16, tag="m1")
                a1b = lv1.tile([P, G, 128], BF16, tag="a1b")
                ch1 = lv1.tile([P, G, 128], BF16, tag="ch1")
                nc.vector.tensor_tensor(
                    out=lsum1, in0=s2[:, 0], in1=s2[:, 1], op=Alu.add)
                nc.vector.tensor_tensor(
                    out=t1b, in0=s2[:, 0], in1=s2[:, 1], op=Alu.subtract)
                nc.vector.tensor_tensor(
                    out=m1, in0=ad2[:, 0], in1=ad2[:, 1], op=Alu.max)
                nc.scalar.activation(out=a1b, in_=t1b, func=Act.Abs, scale=0.5)
                nc.vector.tensor_tensor(
                    out=ch1, in0=a1b, in1=m1, op=Alu.add)

                NF1 = G * 128
                Y_lo = ps0.tile([P, 2, 512], F32, tag="Y_A0")
                Y_h1 = ps0.tile([P, 2, 512], F32, tag="Y_B0")
                for p in range(2):
                    nc.tensor.matmul(
                        out=Y_lo[:, p, 0:NF1], lhsT=wtile[f"w_vq_{p}"][:, :],
                        rhs=lsum1, start=True, stop=True)
                    nc.tensor.matmul(
                        out=Y_h1[:, p, 0:NF1], lhsT=wtile[f"w_vq_{p}"][:, :],
                        rhs=ch1, start=True, stop=True)
                nc.scalar.copy(
                    out=qlo,
                    in_=Y_lo[:, :, 0:NF1].rearrange("r p (g w) -> r p g w", w=128))
                nc.scalar.copy(
                    out=qh1,
                    in_=Y_h1[:, :, 0:NF1].rearrange("r p (g w) -> r p g w", w=128))
                return t

            def stage_c(it, t):
                """level-1 horizontal + finalize + stores."""
                c0 = it * G
                sl = it % 2
                qlo, qh1 = qlo2[:, sl], qh12[:, sl]
                diffL_lo, diffL_h1 = diffL_lo2[:, sl], diffL_h12[:, sl]
                h0x = t["h0x"]

                for q, dl in ((qlo, diffL_lo), (qh1, diffL_h1)):
                    nc.vector.tensor_tensor(
                        out=dl[:, :, :, 1:128], in0=q[:, :, :, 0:127],
                        in1=q[:, :, :, 1:128], op=Alu.subtract)

                lowT = outp.tile([P, G, 2, 256], BF16, tag="lowT")
                highT = outp.tile([P, G, 2, 256], BF16, tag="highT")
                hh = lv1.tile([P, 2, G, 256], BF16, tag="hh")
                for p in range(2):
                    nc.vector.scalar_tensor_tensor(
                        out=lowT[:, :, p, 0:256:2], in0=diffL_lo[:, p, :, 0:128],
                        scalar=0.25, in1=qlo[:, p], op0=Alu.mult, op1=Alu.add)
                    nc.vector.scalar_tensor_tensor(
                        out=lowT[:, :, p, 1:256:2], in0=diffL_lo[:, p, :, 1:129],
                        scalar=-0.25, in1=qlo[:, p], op0=Alu.mult, op1=Alu.add)
                    nc.vector.scalar_tensor_tensor(
                        out=hh[:, p, :, 0:256:2], in0=diffL_h1[:, p, :, 0:128],
                        scalar=0.25, in1=qh1[:, p], op0=Alu.mult, op1=Alu.add)
                    nc.vector.scalar_tensor_tensor(
                        out=hh[:, p, :, 1:256:2], in0=diffL_h1[:, p, :, 1:129],
                        scalar=-0.25, in1=qh1[:, p], op0=Alu.mult, op1=Alu.add)
                nc.vector.tensor_tensor(
                    out=highT.rearrange("r c p w -> r p c w"), in0=hh, in1=h0x,
                    op=Alu.add)

                nc.sync.dma_start(
                    out=low_d[c0:c0 + G].rearrange("c r p w -> r c p w"),
                    in_=lowT)
                nc.sync.dma_start(
                    out=high_d[c0:c0 + G].rearrange("c r p w -> r c p w"),
                    in_=highT)

            # plain order with loads prefetched 2 groups ahead
            xt = {k: stage_load(k) for k in range(3)}
            for it in range(n_iter):
                if it + 3 < n_iter:
                    xt[it + 3] = stage_load(it + 3)
                t = stage_a(it, xt.pop(it))
                stage_b(it, t)
                stage_c(it, t)

    nc.compile()
    _NC_CACHE[key] = nc
    return nc


# ----------------------------------------------------------------------------
# host entry points
# ----------------------------------------------------------------------------

_RUNNER = None


def _get_runner():
    """Builds (once) a cached sharded jit executable over the 8 cores."""
    global _RUNNER
    if _RUNNER is not None:
        return _RUNNER

    import jax
    from jax.sharding import Mesh, PartitionSpec, NamedSharding
    from jax.experimental.shard_map import shard_map
    import concourse.mybir as mybir
    from concourse import bass2jax
    from concourse.bass2jax import _bass_exec_p, partition_id_tensor

    bass2jax.install_neuronx_cc_hook()
    nc = build_nc(C_)

    partition_name = nc.partition_id_tensor.name if nc.partition_id_tensor else None
    in_names, out_names, out_avals = [], [], []
    for alloc in nc.m.functions[0].allocations:
        if not isinstance(alloc, mybir.MemoryLocationSet):
            continue
        name = alloc.memorylocations[0].name
        if alloc.kind == "ExternalInput":
            if name != partition_name:
                in_names.append(name)
        elif alloc.kind == "ExternalOutput":
            out_names.append(name)
            out_avals.append(jax.core.ShapedArray(
                tuple(alloc.tensor_shape), mybir.dt.np(alloc.dtype)))
    n_params = len(in_names)
    all_in_names = list(in_names) + list(out_names)
    if partition_name is not None:
        all_in_names.append(partition_name)

    def _body(*args):
        operands = list(args)
        if partition_name is not None:
            operands.append(partition_id_tensor())
        return tuple(_bass_exec_p.bind(
            *operands,
            out_avals=tuple(out_avals),
            in_names=tuple(all_in_names),
            out_names=tuple(out_names),
            lowering_input_output_aliases=(),
            sim_require_finite=True,
            sim_require_nnan=True,
            nc=nc,
        ))

    devices = jax.devices()[:NCORES]
    mesh = Mesh(np.asarray(devices), ("core",))
    n_in = n_params + len(out_names)
    sharded = jax.jit(shard_map(
        _body, mesh=mesh,
        in_specs=(PartitionSpec("core"),) * n_in,
        out_specs=(PartitionSpec("core"),) * len(out_names),
        check_rep=False))

    shard0 = NamedSharding(mesh, PartitionSpec("core"))
    wt = _weights()
    static = {}
    for name in in_names:
        if name == "x":
            continue
        arr = np.concatenate([wt[name]] * NCORES, axis=0)
        static[name] = jax.device_put(arr, shard0)
    for name, aval in zip(out_names, out_avals):
        z = np.zeros((aval.shape[0] * NCORES,) + tuple(aval.shape[1:]),
                     dtype=aval.dtype)
        static[name] = jax.device_put(z, shard0)

    def run(x_global):
        ops = []
        for name in in_names:
            ops.append(x_global if name == "x" else static[name])
        for name in out_names:
            ops.append(static[name])
        outs = sharded(*ops)
        return dict(zip(out_names, outs))

    _RUNNER = (run, shard0)
    return _RUNNER


def _run_device(x, trace=False):
    """x: [8, 64, 256, 256] fp32. Returns (low, high, results_obj)."""
    if trace:
        from concourse import bass_utils
        nc = build_nc(C_)
        wt = _weights()
        in_maps = [
            dict(wt, x=np.ascontiguousarray(x[b]).reshape(C_, 128, 2, W_))
            for b in range(NCORES)
        ]
        res = bass_utils.run_bass_kernel_spmd(
            nc, in_maps, core_ids=list(range(NCORES)), trace=True)
        low = np.stack([
            res.results[b]["low"].reshape(C_, H_, W_) for b in range(NCORES)])
        high = np.stack([
            res.results[b]["high"].reshape(C_, H_, W_) for b in range(NCORES)])
        return low.astype(np.float32), high.astype(np.float32), res

    run, _ = _get_runner()
    outs = run(np.ascontiguousarray(x).reshape(B_ * C_, 128, 2, W_))
    low = np.asarray(outs["low"]).astype(np.float32).reshape(B_, C_, H_, W_)
    high = np.asarray(outs["high"]).astype(np.float32).reshape(B_, C_, H_, W_)
    return low, high, None


def _fallback(x, level):
    """Numpy port of the reference for unexpected shapes/levels."""
    xl = x.astype(np.float64)
    low = xl
    high = np.zeros_like(xl)
    Bb, Cc, H, W = xl.shape

    def up(a, n_r, n_c):
        Mr = _resize_matrix(a.shape[-2], n_r)
        Mc = _resize_matrix(a.shape[-1], n_c)
        return np.einsum("ij,...jk,lk->...il", Mr, a, Mc)

    for lv in range(level):
        stride = 2 ** lv
        if H // stride < 2 or W // stride < 2:
            break
        x00 = low[..., 0:H - 1:stride, 0:W - 1:stride]
        x01 = low[..., 0:H - 1:stride, 1:W:stride]
        x10 = low[..., 1:H:stride, 0:W - 1:stride]
        x11 = low[..., 1:H:stride, 1:W:stride]
        ll = (x00 + x01 + x10 + x11) * 0.25
        lh = (x00 + x01 - x10 - x11) * 0.25
        hl = (x00 - x01 + x10 - x11) * 0.25
        hh = (x00 - x01 - x10 + x11) * 0.25
        ch = np.abs(lh) + np.abs(hl) + np.abs(hh)
        high = high + up(ch, H, W)
        low = up(ll, H, W)
    if level > 0:
        high = high / level
    return low.astype(np.float32), high.astype(np.float32)


def kernel(x, level):
    x = np.asarray(x, dtype=np.float32)
    level = int(level)
    if level != 2 or x.shape != (B_, C_, H_, W_):
        return _fallback(x, level)
    low, high, _ = _run_device(x)
    return low, high
